# revision 14
# baseline (speedup 1.0000x reference)
"""Trainium2 Bass kernel: causal multi-head self-attention with RoPE.

Problem: B=4, S=2048, D=1024, H=16, DK=64.  out = softmax(causal(qk^T/8)) v @ wo^T
with q,k RoPE-rotated.

Sharding: 8 cores = (batch b in 0..3) x (head-group g in 0..1, 8 heads each).
Each core computes its batch's QKV for its 8 heads, causal attention, and a
partial output projection; the host sums the two group-partials per batch.

Schedule (per core): the PE's only idle source is waiting on ACT (exp) in
attention phases, so all non-attention matmul work is used as filler there:
  upfront: pair-0 q/k projection + RoPE, v projection tiles 0..5.
  pair loop: scores (ST[k,q], K=64, two heads via tile_position row groups)
    -> exp at FD=1024 -> attn@V with ones-augmented V accumulating in PSUM;
    filler dripped between groups: remaining v-proj tiles, next pair's q/k
    proj + rope, and (during pair 3) per-qc output projection as soon as
    that qc's a_t completes.  Normalization per (pair, qc):
    reciprocal_approx_fast + broadcast-DMA.
"""
import os
import sys

for _p in ("/opt/trn_rl_repo", "/root/.axon_site/_ro/trn_rl_repo"):
    if os.path.isdir(_p) and _p not in sys.path:
        sys.path.insert(0, _p)

import numpy as np
import ml_dtypes

import concourse.bass as bass
import concourse.mybir as mybir
import concourse.tile as tile
from concourse import bacc
from concourse.bass_utils import run_bass_kernel_spmd

B, S, D, H = 4, 2048, 1024, 16
DK = D // H          # 64
HG = 8               # heads per group
NG = 2               # head groups (cores per batch)
THETA = 10000.0
NCORES = 8

BF16 = mybir.dt.bfloat16
F32 = mybir.dt.float32
bf16 = ml_dtypes.bfloat16

QT = 512             # q tile width (free dim)
NQT = S // QT        # 4
NKT = S // 128       # 16 k chunks
NJT = HG * DK // 128  # 4 j-tiles (head pairs)
NDC = D // 128       # 8 d chunks
NMT = D // 128       # 8 output m tiles
NVUP = 6             # v-proj tiles emitted before the pair loop


def _build_nc():
    nc = bacc.Bacc("TRN2", target_bir_lowering=False, debug=False)
    xT = nc.dram_tensor("xT", [D, S], BF16, kind="ExternalInput").ap()
    wqT = nc.dram_tensor("wqT", [D, HG * DK], BF16, kind="ExternalInput").ap()
    wkT = nc.dram_tensor("wkT", [D, HG * DK], BF16, kind="ExternalInput").ap()
    wvT = nc.dram_tensor("wvT", [D, HG * DK], BF16, kind="ExternalInput").ap()
    woT = nc.dram_tensor("woT", [HG * DK, D], BF16, kind="ExternalInput").ap()
    c128 = nc.dram_tensor("c128", [128, S], BF16, kind="ExternalInput").ap()
    s128 = nc.dram_tensor("s128", [128, S], BF16, kind="ExternalInput").ap()
    maskd = nc.dram_tensor("maskd", [128, 4, QT], BF16, kind="ExternalInput").ap()
    outT = nc.dram_tensor("outT", [D, S], F32, kind="ExternalOutput").ap()

    from contextlib import ExitStack
    with tile.TileContext(nc) as tc, ExitStack() as stk:
        pp = stk.enter_context(tc.tile_pool(name="persist", bufs=1))
        ep = stk.enter_context(tc.tile_pool(name="epool", bufs=5))
        sp = stk.enter_context(tc.tile_pool(name="smalls", bufs=2))
        qw = stk.enter_context(tc.tile_pool(name="qkvwork", bufs=2))
        ps_st = stk.enter_context(
            tc.tile_pool(name="ps_st", bufs=2, space="PSUM"))
        ps_ov = stk.enter_context(
            tc.tile_pool(name="ps_ov", bufs=2, space="PSUM"))
        ps_qkv = stk.enter_context(
            tc.tile_pool(name="ps_qkv", bufs=2, space="PSUM"))

        # ---------------- persistent tiles ----------------
        wo_sb = pp.tile([128, NJT, D], BF16)
        m_sb = pp.tile([128, 4, QT], BF16)
        qrot = pp.tile([128, NJT, S], BF16)
        krot = pp.tile([128, NJT, S], BF16)
        v_aug = pp.tile([128, NKT, HG, 66], BF16)
        a_t = pp.tile([128, NJT, S], BF16)
        xT_sb = pp.tile([128, NDC, S], BF16)
        wq_sb = pp.tile([128, NDC, HG * DK], BF16)
        wk_sb = pp.tile([128, NDC, HG * DK], BF16)
        wv_sb = pp.tile([128, NDC, HG * DK], BF16)
        c_sb = pp.tile([128, S], BF16)
        s_sb = pp.tile([128, S], BF16)

        nc.gpsimd.memset(v_aug[:, :, :, 64:65], 1.0)

        # ---------------- input DMAs, critical-path first ----------------
        for dc in range(NDC):
            nc.sync.dma_start(xT_sb[:, dc, :], xT[dc * 128:(dc + 1) * 128, :])
        for dc in range(NDC):
            nc.sync.dma_start(wq_sb[:, dc, :], wqT[dc * 128:(dc + 1) * 128, :])
        nc.sync.dma_start(c_sb[:], c128[:])
        nc.sync.dma_start(s_sb[:], s128[:])
        for dc in range(NDC):
            nc.sync.dma_start(wk_sb[:, dc, :], wkT[dc * 128:(dc + 1) * 128, :])
        for dc in range(NDC):
            nc.sync.dma_start(wv_sb[:, dc, :], wvT[dc * 128:(dc + 1) * 128, :])
        nc.sync.dma_start(m_sb[:], maskd[:])
        for jc in range(NJT):
            nc.sync.dma_start(wo_sb[:, jc, :], woT[jc * 128:(jc + 1) * 128, :])

        # ---------------- unit emitters ----------------
        def vproj_unit(tt):
            ps = ps_qkv.tile([128, QT], F32, tag="qv", name=f"psv{tt}")
            for dc in range(NDC):
                nc.tensor.matmul(
                    ps[:],
                    xT_sb[:, dc, tt * 128:(tt + 1) * 128],
                    wv_sb[:, dc, :],
                    start=(dc == 0), stop=(dc == NDC - 1))
            nc.vector.tensor_copy(
                v_aug[:, tt, :, 0:64],
                ps[:].rearrange("p (h d) -> p h d", h=HG))

        def proj_unit(pair, name, w_sb, pre, tn):
            ps = ps_qkv.tile([128, QT], F32, tag="qv",
                             name=f"ps{name}{pair}{tn}")
            for dc in range(NDC):
                nc.tensor.matmul(
                    ps[:],
                    w_sb[:, dc, pair * 128:(pair + 1) * 128],
                    xT_sb[:, dc, tn * QT:(tn + 1) * QT],
                    start=(dc == 0), stop=(dc == NDC - 1))
            nc.vector.tensor_copy(pre[:, tn * QT:(tn + 1) * QT], ps[:])

        def rope_unit(pair, name, pre, dst):
            swp = qw.tile([128, S], BF16, tag="swp", name=f"swp{name}{pair}")
            for a in range(4):
                lo, sw = 32 * a, 32 * (a ^ 1)
                nc.sync.dma_start(swp[lo:lo + 32, :], pre[sw:sw + 32, :])
            nc.vector.tensor_mul(dst[:, pair, :], pre[:], c_sb[:])
            # sin-mul on gpsimd: keeps the loaded Vector queue one op shorter
            nc.gpsimd.tensor_mul(swp[:], swp[:], s_sb[:])
            nc.vector.tensor_add(dst[:, pair, :], dst[:, pair, :], swp[:])

        def emit_scores(pair, qc, g):
            st0 = ps_st.tile([128, 2 * QT], F32, tag="st",
                             name=f"st0_{pair}{qc}{g}")
            st1 = ps_st.tile([128, 2 * QT], F32, tag="st",
                             name=f"st1_{pair}{qc}{g}")
            for half in range(2):
                kc = 2 * g + half
                for h01, st in ((0, st0), (1, st1)):
                    lo = 64 * h01
                    nc.tensor.matmul(
                        st[:, half * QT:(half + 1) * QT],
                        krot[lo:lo + 64, pair, kc * 128:(kc + 1) * 128],
                        qrot[lo:lo + 64, pair, qc * QT:(qc + 1) * QT],
                        start=True, stop=True,
                        tile_position=(lo, 0))
            return st0, st1

        def emit_tail(pair, qc, g, st0, st1, ov0, ov1, last):
            for h01, st, ov in ((0, st0, ov0), (1, st1, ov1)):
                e = ep.tile([128, 2 * QT], BF16, tag="e",
                            name=f"e{pair}{qc}{g}{h01}")
                nc.scalar.activation(
                    e[:], st[:], mybir.ActivationFunctionType.Exp,
                    scale=0.125)
                if g >= 2 * qc:    # diagonal band
                    par = g - 2 * qc
                    e3 = e[:].rearrange("p (a q) -> p a q", a=2)
                    nc.vector.tensor_mul(
                        e3, e3, m_sb[:, 2 * par:2 * par + 2, :])
                for half in range(2):
                    kc = 2 * g + half
                    nc.tensor.matmul(
                        ov[:],
                        v_aug[:, kc, 2 * pair + h01, 0:65],
                        e[:, half * QT:(half + 1) * QT],
                        start=(kc == 0),
                        stop=(last and half == 1))

        def emit_den(pair, qc, ov0, ov1):
            """Stage the denominators straight out of PSUM (no evac copy)."""
            dens = []
            for h01, ov in ((0, ov0), (1, ov1)):
                den = sp.tile([1, QT], F32, tag=f"den{h01}", bufs=3,
                              name=f"den{pair}{qc}{h01}")
                nc.vector.tensor_copy(den[:], ov[64:65, :])
                dens.append(den)
            return dens

        def emit_recip(pair, qc, dens):
            """Part B1: reciprocal + broadcast DMAs (deferred one qc)."""
            rbs = []
            for h01 in range(2):
                recip = sp.tile([1, QT], F32, tag=f"recip{h01}",
                                name=f"rcp{pair}{qc}{h01}")
                nc.vector.reciprocal_approx_fast(recip[:], dens[h01][:])
                rb = sp.tile([64, QT], BF16, tag="rb", bufs=5,
                             name=f"rb{pair}{qc}{h01}")
                nc.gpsimd.dma_start(
                    rb[:],
                    recip[0:1, :]
                    .unsqueeze(1).to_broadcast((1, 64, QT)))
                rbs.append(rb)
            return rbs

        def emit_div(pair, qc, ov0, ov1, rbs):
            """Normalize multiplies straight from PSUM; frees the ov banks."""
            nc.vector.tensor_mul(
                a_t[0:64, pair, qc * QT:(qc + 1) * QT],
                ov0[0:64, :], rbs[0][:])
            an = sp.tile([64, QT], BF16, tag="an", bufs=3,
                         name=f"an{pair}{qc}")
            nc.vector.tensor_mul(an[:], ov1[0:64, :], rbs[1][:])
            nc.sync.dma_start(
                a_t[64:128, pair, qc * QT:(qc + 1) * QT], an[:])

        def outproj_unit(qc, mt):
            op = ps_qkv.tile([128, QT], F32, tag="qv", name=f"op{qc}{mt}")
            for jc in range(NJT):
                nc.tensor.matmul(
                    op[:],
                    wo_sb[:, jc, mt * 128:(mt + 1) * 128],
                    a_t[:, jc, qc * QT:(qc + 1) * QT],
                    start=(jc == 0), stop=(jc == NJT - 1))
            ot = sp.tile([128, QT], F32, tag="ot", bufs=2,
                         name=f"ot{qc}{mt}")
            nc.vector.tensor_copy(ot[:], op[:])
            # scalar queue: keep outT stores off the sync queue, which
            # carries the a_t writes that gate the remaining outproj work
            nc.scalar.dma_start(
                outT[mt * 128:(mt + 1) * 128, qc * QT:(qc + 1) * QT],
                ot[:])

        def proj_units(pair):
            preq = qw.tile([128, S], BF16, tag="preq", name=f"preq{pair}")
            prek = qw.tile([128, S], BF16, tag="prek", name=f"prek{pair}")
            for tn in range(NQT):
                yield lambda tn=tn: proj_unit(pair, "q", wq_sb, preq, tn)
            yield lambda: rope_unit(pair, "q", preq, qrot)
            for tn in range(NQT):
                yield lambda tn=tn: proj_unit(pair, "k", wk_sb, prek, tn)
            yield lambda: rope_unit(pair, "k", prek, krot)

        # ------- upfront: pair-0 q/k proj dc-outer (overlaps input DMA) ---
        preq0 = qw.tile([128, S], BF16, tag="preq", name="preq0")
        prek0 = qw.tile([128, S], BF16, tag="prek", name="prek0")
        for wsb, pre, nm in ((wq_sb, preq0, "q"), (wk_sb, prek0, "k")):
            for tn0 in (0, 2):
                pss = [ps_qkv.tile([128, QT], F32, tag="qv",
                                   name=f"boot{nm}{tn0 + i}")
                       for i in range(2)]
                for dc in range(NDC):
                    for i in range(2):
                        tn = tn0 + i
                        nc.tensor.matmul(
                            pss[i][:],
                            wsb[:, dc, 0:128],
                            xT_sb[:, dc, tn * QT:(tn + 1) * QT],
                            start=(dc == 0), stop=(dc == NDC - 1))
                for i in range(2):
                    nc.vector.tensor_copy(
                        pre[:, (tn0 + i) * QT:(tn0 + i + 1) * QT], pss[i][:])
            rope_unit(0, nm, pre, qrot if nm == "q" else krot)
        for tt in range(NVUP):
            vproj_unit(tt)

        from collections import deque
        filler = deque()
        for tt in range(NVUP, NKT):
            filler.append(lambda tt=tt: vproj_unit(tt))

        # per qc: how many filler units to drip in after each group
        # (placed mid-stream so the scores pipeline stays primed)
        UNIT_BUDGET = {0: 3, 1: 4, 2: 5, 3: 6}
        UNIT_BUDGET_P3 = {0: 2, 1: 4, 2: 6, 3: 8}

        prev_close = [None]   # deferred evac/recip/div of the previous qc

        def close_qc(pair, qc, ov0, ov1):
            def run():
                dens = emit_den(pair, qc, ov0, ov1)
                rbs = emit_recip(pair, qc, dens)
                emit_div(pair, qc, ov0, ov1, rbs)
                if pair == NJT - 1:
                    for mt in range(NMT):
                        filler.append(
                            lambda qc=qc, mt=mt: outproj_unit(qc, mt))
            return run

        for pair in range(NJT):
            if pair + 1 < NJT:
                filler.extend(proj_units(pair + 1))
            budgets = UNIT_BUDGET_P3 if pair == NJT - 1 else UNIT_BUDGET
            for qc in range(NQT):
                ngrp = 2 * qc + 2
                ov0 = ps_ov.tile([65, QT], F32, tag="ov",
                                 name=f"ov0_{pair}{qc}")
                ov1 = ps_ov.tile([65, QT], F32, tag="ov",
                                 name=f"ov1_{pair}{qc}")
                budget = budgets[qc]
                pend = None
                for g in range(ngrp):
                    sts = emit_scores(pair, qc, g)
                    if g == 0 and prev_close[0] is not None:
                        prev_close[0]()      # previous qc's evac/recip/div
                        prev_close[0] = None
                    if pend is not None:
                        pg, p0, p1 = pend
                        emit_tail(pair, qc, pg, p0, p1, ov0, ov1, last=False)
                    pend = (g, sts[0], sts[1])
                    if g >= 1 and budget > 0 and filler:
                        filler.popleft()()
                        budget -= 1
                pg, p0, p1 = pend
                emit_tail(pair, qc, pg, p0, p1, ov0, ov1, last=True)
                prev_close[0] = close_qc(pair, qc, ov0, ov1)

            if pair < NJT - 1:
                while filler:
                    filler.popleft()()

        prev_close[0]()
        while filler:
            filler.popleft()()

    nc.compile()
    return nc


_NC_CACHE = {}


def _get_nc():
    if "nc" not in _NC_CACHE:
        _NC_CACHE["nc"] = _build_nc()
    return _NC_CACHE["nc"]


def _host_prep(x, wq, wk, wv, wo, token_positions):
    head_perm = np.concatenate([np.arange(0, DK, 2), np.arange(1, DK, 2)])
    pos = np.asarray(token_positions).astype(np.float32)
    half = np.arange(0, DK, 2, dtype=np.float32) / DK
    inv_freq = THETA ** (-half)
    ang = pos[:, None] * inv_freq[None, :]        # [S, 32]
    cosT = np.cos(ang).T.astype(np.float32)       # [32, S]
    sinT = np.sin(ang).T.astype(np.float32)
    c128 = np.tile(cosT, (4, 1)).astype(bf16)     # [128, S]
    s128 = np.concatenate([-sinT, sinT, -sinT, sinT], 0).astype(bf16)

    kp = np.arange(128)[:, None, None]
    jj = np.arange(4)[None, :, None]
    qf = np.arange(QT)[None, None, :]
    maskd = (qf >= kp + 128 * jj).astype(bf16)    # [128, 4, QT]

    def prep_qk(w, g):
        rows = w.reshape(H, DK, D)[g * HG:(g + 1) * HG][:, head_perm]
        return np.ascontiguousarray(rows.reshape(HG * DK, D).T).astype(bf16)

    def prep_v(w, g):
        rows = w.reshape(H, DK, D)[g * HG:(g + 1) * HG]
        return np.ascontiguousarray(rows.reshape(HG * DK, D).T).astype(bf16)

    common = {"c128": c128, "s128": s128, "maskd": maskd}
    in_maps = []
    for c in range(NCORES):
        b, g = c // NG, c % NG
        m = dict(common)
        m["xT"] = np.ascontiguousarray(x[b].T).astype(bf16)
        m["wqT"] = prep_qk(wq, g)
        m["wkT"] = prep_qk(wk, g)
        m["wvT"] = prep_v(wv, g)
        m["woT"] = np.ascontiguousarray(wo[:, g * HG * DK:(g + 1) * HG * DK].T
                                        ).astype(bf16)
        in_maps.append(m)
    return in_maps


def kernel(x, wq, wk, wv, wo, token_positions, _trace=False):
    x = np.asarray(x, dtype=np.float32)
    in_maps = _host_prep(x, wq, wk, wv, wo, token_positions)
    nc = _get_nc()
    res = run_bass_kernel_spmd(nc, in_maps, core_ids=list(range(NCORES)),
                               trace=_trace)
    out = np.zeros((B, S, D), np.float32)
    for b in range(B):
        acc = res.results[2 * b]["outT"].astype(np.float32) + \
            res.results[2 * b + 1]["outT"].astype(np.float32)
        out[b] = acc.T
    if _trace:
        kernel.last_results = res
    return out


# revision 15
# speedup vs baseline: 1.1322x; 1.1322x over previous
"""Trainium2 Bass kernel: causal multi-head self-attention with RoPE.

Problem: B=4, S=2048, D=1024, H=16, DK=64.  out = softmax(causal(qk^T/8)) v @ wo^T
with q,k RoPE-rotated.

Sharding: 8 cores = (batch b in 0..3) x (head-group g in 0..1, 8 heads each).
Each core computes its batch's QKV for its 8 heads, causal attention, and a
partial output projection; the host sums the two group-partials per batch.

Schedule (per core): the PE's only idle source is waiting on ACT (exp) in
attention phases, so all non-attention matmul work is used as filler there:
  upfront: pair-0 q/k projection + RoPE, v projection tiles 0..5.
  pair loop: scores (ST[k,q], K=64, two heads via tile_position row groups)
    -> exp at FD=1024 -> attn@V with ones-augmented V accumulating in PSUM;
    filler dripped between groups: remaining v-proj tiles, next pair's q/k
    proj + rope, and (during pair 3) per-qc output projection as soon as
    that qc's a_t completes.  Normalization per (pair, qc):
    reciprocal_approx_fast + broadcast-DMA.
"""
import os
import sys

for _p in ("/opt/trn_rl_repo", "/root/.axon_site/_ro/trn_rl_repo"):
    if os.path.isdir(_p) and _p not in sys.path:
        sys.path.insert(0, _p)

import numpy as np
import ml_dtypes

import concourse.bass as bass
import concourse.mybir as mybir
import concourse.tile as tile
from concourse import bacc
from concourse.bass_utils import run_bass_kernel_spmd

B, S, D, H = 4, 2048, 1024, 16
DK = D // H          # 64
HG = 8               # heads per group
NG = 2               # head groups (cores per batch)
THETA = 10000.0
NCORES = 8

BF16 = mybir.dt.bfloat16
F32 = mybir.dt.float32
bf16 = ml_dtypes.bfloat16

QT = 512             # q tile width (free dim)
NQT = S // QT        # 4
NKT = S // 128       # 16 k chunks
NJT = HG * DK // 128  # 4 j-tiles (head pairs)
NDC = D // 128       # 8 d chunks
NMT = D // 128       # 8 output m tiles
NVUP = 6             # v-proj tiles emitted before the pair loop


def _build_nc():
    nc = bacc.Bacc("TRN2", target_bir_lowering=False, debug=False)
    xT = nc.dram_tensor("xT", [D, S], BF16, kind="ExternalInput").ap()
    wqT = nc.dram_tensor("wqT", [D, HG * DK], BF16, kind="ExternalInput").ap()
    wkT = nc.dram_tensor("wkT", [D, HG * DK], BF16, kind="ExternalInput").ap()
    wvT = nc.dram_tensor("wvT", [D, HG * DK], BF16, kind="ExternalInput").ap()
    woT = nc.dram_tensor("woT", [HG * DK, D], BF16, kind="ExternalInput").ap()
    c128 = nc.dram_tensor("c128", [128, S], BF16, kind="ExternalInput").ap()
    s128 = nc.dram_tensor("s128", [128, S], BF16, kind="ExternalInput").ap()
    maskd = nc.dram_tensor("maskd", [128, 4, QT], BF16, kind="ExternalInput").ap()
    outT = nc.dram_tensor("outT", [D, S], F32, kind="ExternalOutput").ap()

    from contextlib import ExitStack
    with tile.TileContext(nc) as tc, ExitStack() as stk:
        pp = stk.enter_context(tc.tile_pool(name="persist", bufs=1))
        ep = stk.enter_context(tc.tile_pool(name="epool", bufs=6))
        sp = stk.enter_context(tc.tile_pool(name="smalls", bufs=2))
        qw = stk.enter_context(tc.tile_pool(name="qkvwork", bufs=2))
        ps_st = stk.enter_context(
            tc.tile_pool(name="ps_st", bufs=2, space="PSUM"))
        ps_ov = stk.enter_context(
            tc.tile_pool(name="ps_ov", bufs=2, space="PSUM"))
        ps_qkv = stk.enter_context(
            tc.tile_pool(name="ps_qkv", bufs=2, space="PSUM"))

        # ---------------- persistent tiles ----------------
        wo_sb = pp.tile([128, NJT, D], BF16)
        m_sb = pp.tile([128, 4, QT], BF16)
        qrot = pp.tile([128, NJT, S], BF16)
        krot = pp.tile([128, NJT, S], BF16)
        v_aug = pp.tile([128, NKT, HG, 66], BF16)
        a_t = pp.tile([128, NJT, S], BF16)
        xT_sb = pp.tile([128, NDC, S], BF16)
        wq_sb = pp.tile([128, NDC, HG * DK], BF16)
        wk_sb = pp.tile([128, NDC, HG * DK], BF16)
        wv_sb = pp.tile([128, NDC, HG * DK], BF16)
        c_sb = pp.tile([128, S], BF16)
        s_sb = pp.tile([128, S], BF16)

        nc.gpsimd.memset(v_aug[:, :, :, 64:65], 1.0)

        # ---------------- input DMAs, critical-path first ----------------
        for dc in range(NDC):
            nc.sync.dma_start(xT_sb[:, dc, :], xT[dc * 128:(dc + 1) * 128, :])
        for dc in range(NDC):
            nc.sync.dma_start(wq_sb[:, dc, :], wqT[dc * 128:(dc + 1) * 128, :])
        nc.sync.dma_start(c_sb[:], c128[:])
        nc.sync.dma_start(s_sb[:], s128[:])
        for dc in range(NDC):
            nc.sync.dma_start(wk_sb[:, dc, :], wkT[dc * 128:(dc + 1) * 128, :])
        for dc in range(NDC):
            nc.sync.dma_start(wv_sb[:, dc, :], wvT[dc * 128:(dc + 1) * 128, :])
        nc.sync.dma_start(m_sb[:], maskd[:])
        for jc in range(NJT):
            nc.sync.dma_start(wo_sb[:, jc, :], woT[jc * 128:(jc + 1) * 128, :])

        # ---------------- unit emitters ----------------
        def vproj_unit(tt):
            ps = ps_qkv.tile([128, QT], F32, tag="qv", name=f"psv{tt}")
            for dc in range(NDC):
                nc.tensor.matmul(
                    ps[:],
                    xT_sb[:, dc, tt * 128:(tt + 1) * 128],
                    wv_sb[:, dc, :],
                    start=(dc == 0), stop=(dc == NDC - 1))
            nc.vector.tensor_copy(
                v_aug[:, tt, :, 0:64],
                ps[:].rearrange("p (h d) -> p h d", h=HG))

        def proj_unit(pair, name, w_sb, pre, tn):
            ps = ps_qkv.tile([128, QT], F32, tag="qv",
                             name=f"ps{name}{pair}{tn}")
            for dc in range(NDC):
                nc.tensor.matmul(
                    ps[:],
                    w_sb[:, dc, pair * 128:(pair + 1) * 128],
                    xT_sb[:, dc, tn * QT:(tn + 1) * QT],
                    start=(dc == 0), stop=(dc == NDC - 1))
            nc.vector.tensor_copy(pre[:, tn * QT:(tn + 1) * QT], ps[:])

        def rope_unit(pair, name, pre, dst):
            swp = qw.tile([128, S], BF16, tag="swp", name=f"swp{name}{pair}")
            for a in range(4):
                lo, sw = 32 * a, 32 * (a ^ 1)
                nc.sync.dma_start(swp[lo:lo + 32, :], pre[sw:sw + 32, :])
            nc.vector.tensor_mul(dst[:, pair, :], pre[:], c_sb[:])
            nc.vector.tensor_mul(swp[:], swp[:], s_sb[:])
            nc.vector.tensor_add(dst[:, pair, :], dst[:, pair, :], swp[:])

        def emit_scores(pair, qc, g):
            st0 = ps_st.tile([128, 2 * QT], F32, tag="st",
                             name=f"st0_{pair}{qc}{g}")
            st1 = ps_st.tile([128, 2 * QT], F32, tag="st",
                             name=f"st1_{pair}{qc}{g}")
            for half in range(2):
                kc = 2 * g + half
                for h01, st in ((0, st0), (1, st1)):
                    lo = 64 * h01
                    nc.tensor.matmul(
                        st[:, half * QT:(half + 1) * QT],
                        krot[lo:lo + 64, pair, kc * 128:(kc + 1) * 128],
                        qrot[lo:lo + 64, pair, qc * QT:(qc + 1) * QT],
                        start=True, stop=True,
                        tile_position=(lo, 0))
            return st0, st1

        def emit_tail(pair, qc, g, st0, st1, ov0, ov1, last):
            for h01, st, ov in ((0, st0, ov0), (1, st1, ov1)):
                e = ep.tile([128, 2 * QT], BF16, tag="e",
                            name=f"e{pair}{qc}{g}{h01}")
                nc.scalar.activation(
                    e[:], st[:], mybir.ActivationFunctionType.Exp,
                    scale=0.125)
                if g >= 2 * qc:    # diagonal band
                    par = g - 2 * qc
                    e3 = e[:].rearrange("p (a q) -> p a q", a=2)
                    nc.vector.tensor_mul(
                        e3, e3, m_sb[:, 2 * par:2 * par + 2, :])
                for half in range(2):
                    kc = 2 * g + half
                    nc.tensor.matmul(
                        ov[:],
                        v_aug[:, kc, 2 * pair + h01, 0:65],
                        e[:, half * QT:(half + 1) * QT],
                        start=(kc == 0),
                        stop=(last and half == 1))

        def emit_evac(pair, qc, ov0, ov1):
            """Part A: free the ov PSUM banks and stage the denominators."""
            den = sp.tile([2, QT], F32, tag="den", bufs=3,
                          name=f"den{pair}{qc}")
            ous = []
            for h01, ov in ((0, ov0), (1, ov1)):
                ou = ep.tile([65, QT], BF16, tag="ou", bufs=6,
                             name=f"ou{pair}{qc}{h01}")
                nc.vector.tensor_copy(ou[:], ov[:])
                nc.gpsimd.dma_start(den[h01:h01 + 1, :], ou[64:65, :])
                ous.append(ou)
            return den, ous

        def emit_recip(pair, qc, den):
            """Part B1: reciprocal + broadcast DMAs (deferred one qc)."""
            recip = sp.tile([2, QT], F32, tag="recip", name=f"rcp{pair}{qc}")
            nc.vector.reciprocal_approx_fast(recip[:], den[:])
            rbs = []
            for h01 in range(2):
                rb = sp.tile([64, QT], BF16, tag="rb", bufs=5,
                             name=f"rb{pair}{qc}{h01}")
                nc.gpsimd.dma_start(
                    rb[:],
                    recip[h01:h01 + 1, :]
                    .unsqueeze(1).to_broadcast((1, 64, QT)))
                rbs.append(rb)
            return rbs

        def emit_div(pair, qc, ous, rbs):
            """Part B2: the normalize multiplies (deferred further)."""
            nc.vector.tensor_mul(
                a_t[0:64, pair, qc * QT:(qc + 1) * QT],
                ous[0][0:64, :], rbs[0][:])
            an = sp.tile([64, QT], BF16, tag="an", bufs=3,
                         name=f"an{pair}{qc}")
            nc.vector.tensor_mul(an[:], ous[1][0:64, :], rbs[1][:])
            nc.sync.dma_start(
                a_t[64:128, pair, qc * QT:(qc + 1) * QT], an[:])

        def outproj_unit(qc, mt):
            op = ps_qkv.tile([128, QT], F32, tag="qv", name=f"op{qc}{mt}")
            for jc in range(NJT):
                nc.tensor.matmul(
                    op[:],
                    wo_sb[:, jc, mt * 128:(mt + 1) * 128],
                    a_t[:, jc, qc * QT:(qc + 1) * QT],
                    start=(jc == 0), stop=(jc == NJT - 1))
            ot = sp.tile([128, QT], F32, tag="ot", bufs=3,
                         name=f"ot{qc}{mt}")
            nc.vector.tensor_copy(ot[:], op[:])
            # scalar queue: keep outT stores off the sync queue, which
            # carries the a_t writes that gate the remaining outproj work
            nc.scalar.dma_start(
                outT[mt * 128:(mt + 1) * 128, qc * QT:(qc + 1) * QT],
                ot[:])

        def proj_units(pair):
            preq = qw.tile([128, S], BF16, tag="preq", name=f"preq{pair}")
            prek = qw.tile([128, S], BF16, tag="prek", name=f"prek{pair}")
            for tn in range(NQT):
                yield lambda tn=tn: proj_unit(pair, "q", wq_sb, preq, tn)
            yield lambda: rope_unit(pair, "q", preq, qrot)
            for tn in range(NQT):
                yield lambda tn=tn: proj_unit(pair, "k", wk_sb, prek, tn)
            yield lambda: rope_unit(pair, "k", prek, krot)

        # ---------------- upfront: pair-0 q/k + rope, v tiles 0..5 --------
        for u in proj_units(0):
            u()
        for tt in range(NVUP):
            vproj_unit(tt)

        from collections import deque
        filler = deque()
        for tt in range(NVUP, NKT):
            filler.append(lambda tt=tt: vproj_unit(tt))

        # per qc: how many filler units to drip in after each group
        # (placed mid-stream so the scores pipeline stays primed)
        UNIT_BUDGET = {0: 2, 1: 3, 2: 4, 3: 5}
        UNIT_BUDGET_P3 = {0: 2, 1: 4, 2: 6, 3: 8}


        for pair in range(NJT):
            if pair + 1 < NJT:
                filler.extend(proj_units(pair + 1))
            budgets = UNIT_BUDGET_P3 if pair == NJT - 1 else UNIT_BUDGET
            for qc in range(NQT):
                ngrp = 2 * qc + 2
                ov0 = ps_ov.tile([65, QT], F32, tag="ov",
                                 name=f"ov0_{pair}{qc}")
                ov1 = ps_ov.tile([65, QT], F32, tag="ov",
                                 name=f"ov1_{pair}{qc}")
                budget = budgets[qc]
                pend = None
                for g in range(ngrp):
                    sts = emit_scores(pair, qc, g)
                    if pend is not None:
                        pg, p0, p1 = pend
                        emit_tail(pair, qc, pg, p0, p1, ov0, ov1, last=False)
                    pend = (g, sts[0], sts[1])
                    if g >= 1 and budget > 0 and filler:
                        filler.popleft()()
                        budget -= 1
                pg, p0, p1 = pend
                emit_tail(pair, qc, pg, p0, p1, ov0, ov1, last=True)
                den, ous = emit_evac(pair, qc, ov0, ov1)
                rbs = emit_recip(pair, qc, den)
                emit_div(pair, qc, ous, rbs)
                if pair == NJT - 1:
                    for mt in range(NMT):
                        filler.append(
                            lambda qc=qc, mt=mt: outproj_unit(qc, mt))

            if pair < NJT - 1:
                while filler:
                    filler.popleft()()

        while filler:
            filler.popleft()()

    nc.compile()
    return nc


_NC_CACHE = {}


def _get_nc():
    if "nc" not in _NC_CACHE:
        _NC_CACHE["nc"] = _build_nc()
    return _NC_CACHE["nc"]


def _host_prep(x, wq, wk, wv, wo, token_positions):
    head_perm = np.concatenate([np.arange(0, DK, 2), np.arange(1, DK, 2)])
    pos = np.asarray(token_positions).astype(np.float32)
    half = np.arange(0, DK, 2, dtype=np.float32) / DK
    inv_freq = THETA ** (-half)
    ang = pos[:, None] * inv_freq[None, :]        # [S, 32]
    cosT = np.cos(ang).T.astype(np.float32)       # [32, S]
    sinT = np.sin(ang).T.astype(np.float32)
    c128 = np.tile(cosT, (4, 1)).astype(bf16)     # [128, S]
    s128 = np.concatenate([-sinT, sinT, -sinT, sinT], 0).astype(bf16)

    kp = np.arange(128)[:, None, None]
    jj = np.arange(4)[None, :, None]
    qf = np.arange(QT)[None, None, :]
    maskd = (qf >= kp + 128 * jj).astype(bf16)    # [128, 4, QT]

    def prep_qk(w, g):
        rows = w.reshape(H, DK, D)[g * HG:(g + 1) * HG][:, head_perm]
        return np.ascontiguousarray(rows.reshape(HG * DK, D).T).astype(bf16)

    def prep_v(w, g):
        rows = w.reshape(H, DK, D)[g * HG:(g + 1) * HG]
        return np.ascontiguousarray(rows.reshape(HG * DK, D).T).astype(bf16)

    common = {"c128": c128, "s128": s128, "maskd": maskd}
    in_maps = []
    for c in range(NCORES):
        b, g = c // NG, c % NG
        m = dict(common)
        m["xT"] = np.ascontiguousarray(x[b].T).astype(bf16)
        m["wqT"] = prep_qk(wq, g)
        m["wkT"] = prep_qk(wk, g)
        m["wvT"] = prep_v(wv, g)
        m["woT"] = np.ascontiguousarray(wo[:, g * HG * DK:(g + 1) * HG * DK].T
                                        ).astype(bf16)
        in_maps.append(m)
    return in_maps


def kernel(x, wq, wk, wv, wo, token_positions, _trace=False):
    x = np.asarray(x, dtype=np.float32)
    in_maps = _host_prep(x, wq, wk, wv, wo, token_positions)
    nc = _get_nc()
    res = run_bass_kernel_spmd(nc, in_maps, core_ids=list(range(NCORES)),
                               trace=_trace)
    out = np.zeros((B, S, D), np.float32)
    for b in range(B):
        acc = res.results[2 * b]["outT"].astype(np.float32) + \
            res.results[2 * b + 1]["outT"].astype(np.float32)
        out[b] = acc.T
    if _trace:
        kernel.last_results = res
    return out


# revision 16
# speedup vs baseline: 1.1472x; 1.0132x over previous
"""Trainium2 Bass kernel: causal multi-head self-attention with RoPE.

Problem: B=4, S=2048, D=1024, H=16, DK=64.  out = softmax(causal(qk^T/8)) v @ wo^T
with q,k RoPE-rotated.

Sharding: 8 cores = (batch b in 0..3) x (head-group g in 0..1, 8 heads each).
Each core computes its batch's QKV for its 8 heads, causal attention, and a
partial output projection; the host sums the two group-partials per batch.

Schedule (per core): the PE's only idle source is waiting on ACT (exp) in
attention phases, so all non-attention matmul work is used as filler there:
  upfront: pair-0 q/k projection + RoPE, v projection tiles 0..5.
  pair loop: scores (ST[k,q], K=64, two heads via tile_position row groups)
    -> exp at FD=1024 -> attn@V with ones-augmented V accumulating in PSUM;
    filler dripped between groups: remaining v-proj tiles, next pair's q/k
    proj + rope, and (during pair 3) per-qc output projection as soon as
    that qc's a_t completes.  Normalization per (pair, qc):
    reciprocal_approx_fast + broadcast-DMA.
"""
import os
import sys

for _p in ("/opt/trn_rl_repo", "/root/.axon_site/_ro/trn_rl_repo"):
    if os.path.isdir(_p) and _p not in sys.path:
        sys.path.insert(0, _p)

import numpy as np
import ml_dtypes

import concourse.bass as bass
import concourse.mybir as mybir
import concourse.tile as tile
from concourse import bacc
from concourse.bass_utils import run_bass_kernel_spmd

B, S, D, H = 4, 2048, 1024, 16
DK = D // H          # 64
HG = 8               # heads per group
NG = 2               # head groups (cores per batch)
THETA = 10000.0
NCORES = 8

BF16 = mybir.dt.bfloat16
F32 = mybir.dt.float32
bf16 = ml_dtypes.bfloat16

QT = 512             # q tile width (free dim)
NQT = S // QT        # 4
NKT = S // 128       # 16 k chunks
NJT = HG * DK // 128  # 4 j-tiles (head pairs)
NDC = D // 128       # 8 d chunks
NMT = D // 128       # 8 output m tiles
NVUP = 6             # v-proj tiles emitted before the pair loop


def _build_nc():
    nc = bacc.Bacc("TRN2", target_bir_lowering=False, debug=False)
    xT = nc.dram_tensor("xT", [D, S], BF16, kind="ExternalInput").ap()
    wqT = nc.dram_tensor("wqT", [D, HG * DK], BF16, kind="ExternalInput").ap()
    wkT = nc.dram_tensor("wkT", [D, HG * DK], BF16, kind="ExternalInput").ap()
    wvT = nc.dram_tensor("wvT", [D, HG * DK], BF16, kind="ExternalInput").ap()
    woT = nc.dram_tensor("woT", [HG * DK, D], BF16, kind="ExternalInput").ap()
    c128 = nc.dram_tensor("c128", [128, S], BF16, kind="ExternalInput").ap()
    s128 = nc.dram_tensor("s128", [128, S], BF16, kind="ExternalInput").ap()
    maskd = nc.dram_tensor("maskd", [128, 4, QT], BF16, kind="ExternalInput").ap()
    outT = nc.dram_tensor("outT", [D, S], F32, kind="ExternalOutput").ap()

    from contextlib import ExitStack
    with tile.TileContext(nc) as tc, ExitStack() as stk:
        pp = stk.enter_context(tc.tile_pool(name="persist", bufs=1))
        ep = stk.enter_context(tc.tile_pool(name="epool", bufs=6))
        sp = stk.enter_context(tc.tile_pool(name="smalls", bufs=2))
        qw = stk.enter_context(tc.tile_pool(name="qkvwork", bufs=2))
        ps_st = stk.enter_context(
            tc.tile_pool(name="ps_st", bufs=2, space="PSUM"))
        ps_ov = stk.enter_context(
            tc.tile_pool(name="ps_ov", bufs=2, space="PSUM"))
        ps_qkv = stk.enter_context(
            tc.tile_pool(name="ps_qkv", bufs=2, space="PSUM"))

        # ---------------- persistent tiles ----------------
        wo_sb = pp.tile([128, NJT, D], BF16)
        m_sb = pp.tile([128, 4, QT], BF16)
        qrot = pp.tile([128, NJT, S], BF16)
        krot = pp.tile([128, NJT, S], BF16)
        v_aug = pp.tile([128, NKT, HG, 66], BF16)
        a_t = pp.tile([128, NJT, S], BF16)
        xT_sb = pp.tile([128, NDC, S], BF16)
        wq_sb = pp.tile([128, NDC, HG * DK], BF16)
        wk_sb = pp.tile([128, NDC, HG * DK], BF16)
        wv_sb = pp.tile([128, NDC, HG * DK], BF16)
        c_sb = pp.tile([128, S], BF16)
        s_sb = pp.tile([128, S], BF16)

        nc.gpsimd.memset(v_aug[:, :, :, 64:65], 1.0)

        # ---------------- input DMAs, critical-path first ----------------
        for dc in range(NDC):
            nc.sync.dma_start(xT_sb[:, dc, :], xT[dc * 128:(dc + 1) * 128, :])
        for dc in range(NDC):
            nc.sync.dma_start(wq_sb[:, dc, :], wqT[dc * 128:(dc + 1) * 128, :])
        nc.sync.dma_start(c_sb[:], c128[:])
        nc.sync.dma_start(s_sb[:], s128[:])
        for dc in range(NDC):
            nc.sync.dma_start(wk_sb[:, dc, :], wkT[dc * 128:(dc + 1) * 128, :])
        for dc in range(NDC):
            nc.sync.dma_start(wv_sb[:, dc, :], wvT[dc * 128:(dc + 1) * 128, :])
        nc.sync.dma_start(m_sb[:], maskd[:])
        for jc in range(NJT):
            nc.sync.dma_start(wo_sb[:, jc, :], woT[jc * 128:(jc + 1) * 128, :])

        # ---------------- unit emitters ----------------
        def vproj_unit(tt):
            ps = ps_qkv.tile([128, QT], F32, tag="qv", name=f"psv{tt}")
            for dc in range(NDC):
                nc.tensor.matmul(
                    ps[:],
                    xT_sb[:, dc, tt * 128:(tt + 1) * 128],
                    wv_sb[:, dc, :],
                    start=(dc == 0), stop=(dc == NDC - 1))
            nc.vector.tensor_copy(
                v_aug[:, tt, :, 0:64],
                ps[:].rearrange("p (h d) -> p h d", h=HG))

        def proj_unit(pair, name, w_sb, pre, tn):
            ps = ps_qkv.tile([128, QT], F32, tag="qv",
                             name=f"ps{name}{pair}{tn}")
            for dc in range(NDC):
                nc.tensor.matmul(
                    ps[:],
                    w_sb[:, dc, pair * 128:(pair + 1) * 128],
                    xT_sb[:, dc, tn * QT:(tn + 1) * QT],
                    start=(dc == 0), stop=(dc == NDC - 1))
            nc.vector.tensor_copy(pre[:, tn * QT:(tn + 1) * QT], ps[:])

        def rope_unit(pair, name, pre, dst):
            swp = qw.tile([128, S], BF16, tag="swp", name=f"swp{name}{pair}")
            for a in range(4):
                lo, sw = 32 * a, 32 * (a ^ 1)
                nc.sync.dma_start(swp[lo:lo + 32, :], pre[sw:sw + 32, :])
            nc.vector.tensor_mul(dst[:, pair, :], pre[:], c_sb[:])
            nc.vector.tensor_mul(swp[:], swp[:], s_sb[:])
            nc.vector.tensor_add(dst[:, pair, :], dst[:, pair, :], swp[:])

        def _diag_off(qc, g, half):
            """first unmasked q column of this kc chunk within the q tile
            (0 for interior chunks; 128*j for the j-th diagonal chunk)"""
            par = g - 2 * qc
            if par < 0:
                return 0
            return 128 * (2 * par + half)

        def emit_scores(pair, qc, g):
            st0 = ps_st.tile([128, 2 * QT], F32, tag="st",
                             name=f"st0_{pair}{qc}{g}")
            st1 = ps_st.tile([128, 2 * QT], F32, tag="st",
                             name=f"st1_{pair}{qc}{g}")
            for half in range(2):
                kc = 2 * g + half
                off = _diag_off(qc, g, half)
                for h01, st in ((0, st0), (1, st1)):
                    lo = 64 * h01
                    nc.tensor.matmul(
                        st[:, half * QT + off:(half + 1) * QT],
                        krot[lo:lo + 64, pair, kc * 128:(kc + 1) * 128],
                        qrot[lo:lo + 64, pair, qc * QT + off:(qc + 1) * QT],
                        start=True, stop=True,
                        tile_position=(lo, 0))
            return st0, st1

        def emit_tail(pair, qc, g, st0, st1, ov0, ov1, last):
            par = g - 2 * qc
            for h01, st, ov in ((0, st0, ov0), (1, st1, ov1)):
                e = ep.tile([128, 2 * QT], BF16, tag="e",
                            name=f"e{pair}{qc}{g}{h01}")
                if par < 0:
                    nc.scalar.activation(
                        e[:], st[:], mybir.ActivationFunctionType.Exp,
                        scale=0.125)
                else:
                    # diagonal: exp only the unmasked q ranges, then mask
                    # just the 128-wide triangle band of each kc chunk
                    for half in range(2):
                        off = _diag_off(qc, g, half)
                        sl = slice(half * QT + off, (half + 1) * QT)
                        nc.scalar.activation(
                            e[:, sl], st[:, sl],
                            mybir.ActivationFunctionType.Exp, scale=0.125)
                        nc.vector.tensor_mul(
                            e[:, half * QT + off:half * QT + off + 128],
                            e[:, half * QT + off:half * QT + off + 128],
                            m_sb[:, 2 * par + half, off:off + 128])
                for half in range(2):
                    kc = 2 * g + half
                    off = _diag_off(qc, g, half)
                    nc.tensor.matmul(
                        ov[:, off:QT] if off else ov[:],
                        v_aug[:, kc, 2 * pair + h01, 0:65],
                        e[:, half * QT + off:(half + 1) * QT],
                        start=(kc == 0),
                        stop=(last and half == 1))

        def emit_evac(pair, qc, ov0, ov1):
            """Part A: free the ov PSUM banks and stage the denominators."""
            den = sp.tile([2, QT], F32, tag="den", bufs=3,
                          name=f"den{pair}{qc}")
            ous = []
            for h01, ov in ((0, ov0), (1, ov1)):
                ou = ep.tile([65, QT], BF16, tag="ou", bufs=6,
                             name=f"ou{pair}{qc}{h01}")
                nc.vector.tensor_copy(ou[:], ov[:])
                nc.gpsimd.dma_start(den[h01:h01 + 1, :], ou[64:65, :])
                ous.append(ou)
            return den, ous

        def emit_recip(pair, qc, den):
            """Part B1: reciprocal + broadcast DMAs (deferred one qc)."""
            recip = sp.tile([2, QT], F32, tag="recip", name=f"rcp{pair}{qc}")
            nc.vector.reciprocal_approx_fast(recip[:], den[:])
            rbs = []
            for h01 in range(2):
                rb = sp.tile([64, QT], BF16, tag="rb", bufs=5,
                             name=f"rb{pair}{qc}{h01}")
                nc.gpsimd.dma_start(
                    rb[:],
                    recip[h01:h01 + 1, :]
                    .unsqueeze(1).to_broadcast((1, 64, QT)))
                rbs.append(rb)
            return rbs

        def emit_div(pair, qc, ous, rbs):
            """Part B2: the normalize multiplies (deferred further)."""
            nc.vector.tensor_mul(
                a_t[0:64, pair, qc * QT:(qc + 1) * QT],
                ous[0][0:64, :], rbs[0][:])
            an = sp.tile([64, QT], BF16, tag="an", bufs=3,
                         name=f"an{pair}{qc}")
            nc.vector.tensor_mul(an[:], ous[1][0:64, :], rbs[1][:])
            nc.sync.dma_start(
                a_t[64:128, pair, qc * QT:(qc + 1) * QT], an[:])

        def outproj_unit(qc, mt):
            op = ps_qkv.tile([128, QT], F32, tag="qv", name=f"op{qc}{mt}")
            for jc in range(NJT):
                nc.tensor.matmul(
                    op[:],
                    wo_sb[:, jc, mt * 128:(mt + 1) * 128],
                    a_t[:, jc, qc * QT:(qc + 1) * QT],
                    start=(jc == 0), stop=(jc == NJT - 1))
            ot = sp.tile([128, QT], F32, tag="ot", bufs=3,
                         name=f"ot{qc}{mt}")
            nc.vector.tensor_copy(ot[:], op[:])
            # scalar queue: keep outT stores off the sync queue, which
            # carries the a_t writes that gate the remaining outproj work
            nc.scalar.dma_start(
                outT[mt * 128:(mt + 1) * 128, qc * QT:(qc + 1) * QT],
                ot[:])

        def proj_units(pair):
            preq = qw.tile([128, S], BF16, tag="preq", name=f"preq{pair}")
            prek = qw.tile([128, S], BF16, tag="prek", name=f"prek{pair}")
            for tn in range(NQT):
                yield lambda tn=tn: proj_unit(pair, "q", wq_sb, preq, tn)
            yield lambda: rope_unit(pair, "q", preq, qrot)
            for tn in range(NQT):
                yield lambda tn=tn: proj_unit(pair, "k", wk_sb, prek, tn)
            yield lambda: rope_unit(pair, "k", prek, krot)

        # ---------------- upfront: pair-0 q/k + rope, v tiles 0..5 --------
        for u in proj_units(0):
            u()
        for tt in range(NVUP):
            vproj_unit(tt)

        from collections import deque
        filler = deque()
        for tt in range(NVUP, NKT):
            filler.append(lambda tt=tt: vproj_unit(tt))

        # per qc: how many filler units to drip in after each group
        # (placed mid-stream so the scores pipeline stays primed)
        UNIT_BUDGET = {0: 2, 1: 3, 2: 4, 3: 5}
        UNIT_BUDGET_P3 = {0: 2, 1: 4, 2: 6, 3: 8}


        for pair in range(NJT):
            if pair + 1 < NJT:
                filler.extend(proj_units(pair + 1))
            budgets = UNIT_BUDGET_P3 if pair == NJT - 1 else UNIT_BUDGET
            for qc in range(NQT):
                ngrp = 2 * qc + 2
                ov0 = ps_ov.tile([65, QT], F32, tag="ov",
                                 name=f"ov0_{pair}{qc}")
                ov1 = ps_ov.tile([65, QT], F32, tag="ov",
                                 name=f"ov1_{pair}{qc}")
                budget = budgets[qc]
                pend = None
                for g in range(ngrp):
                    sts = emit_scores(pair, qc, g)
                    if pend is not None:
                        pg, p0, p1 = pend
                        emit_tail(pair, qc, pg, p0, p1, ov0, ov1, last=False)
                    pend = (g, sts[0], sts[1])
                    if g >= 1 and budget > 0 and filler:
                        filler.popleft()()
                        budget -= 1
                pg, p0, p1 = pend
                emit_tail(pair, qc, pg, p0, p1, ov0, ov1, last=True)
                den, ous = emit_evac(pair, qc, ov0, ov1)
                rbs = emit_recip(pair, qc, den)
                emit_div(pair, qc, ous, rbs)
                if pair == NJT - 1:
                    for mt in range(NMT):
                        filler.append(
                            lambda qc=qc, mt=mt: outproj_unit(qc, mt))

            if pair < NJT - 1:
                while filler:
                    filler.popleft()()

        while filler:
            filler.popleft()()

    nc.compile()
    return nc


_NC_CACHE = {}


def _get_nc():
    if "nc" not in _NC_CACHE:
        _NC_CACHE["nc"] = _build_nc()
    return _NC_CACHE["nc"]


def _host_prep(x, wq, wk, wv, wo, token_positions):
    head_perm = np.concatenate([np.arange(0, DK, 2), np.arange(1, DK, 2)])
    pos = np.asarray(token_positions).astype(np.float32)
    half = np.arange(0, DK, 2, dtype=np.float32) / DK
    inv_freq = THETA ** (-half)
    ang = pos[:, None] * inv_freq[None, :]        # [S, 32]
    cosT = np.cos(ang).T.astype(np.float32)       # [32, S]
    sinT = np.sin(ang).T.astype(np.float32)
    c128 = np.tile(cosT, (4, 1)).astype(bf16)     # [128, S]
    s128 = np.concatenate([-sinT, sinT, -sinT, sinT], 0).astype(bf16)

    kp = np.arange(128)[:, None, None]
    jj = np.arange(4)[None, :, None]
    qf = np.arange(QT)[None, None, :]
    maskd = (qf >= kp + 128 * jj).astype(bf16)    # [128, 4, QT]

    def prep_qk(w, g):
        rows = w.reshape(H, DK, D)[g * HG:(g + 1) * HG][:, head_perm]
        return np.ascontiguousarray(rows.reshape(HG * DK, D).T).astype(bf16)

    def prep_v(w, g):
        rows = w.reshape(H, DK, D)[g * HG:(g + 1) * HG]
        return np.ascontiguousarray(rows.reshape(HG * DK, D).T).astype(bf16)

    common = {"c128": c128, "s128": s128, "maskd": maskd}
    in_maps = []
    for c in range(NCORES):
        b, g = c // NG, c % NG
        m = dict(common)
        m["xT"] = np.ascontiguousarray(x[b].T).astype(bf16)
        m["wqT"] = prep_qk(wq, g)
        m["wkT"] = prep_qk(wk, g)
        m["wvT"] = prep_v(wv, g)
        m["woT"] = np.ascontiguousarray(wo[:, g * HG * DK:(g + 1) * HG * DK].T
                                        ).astype(bf16)
        in_maps.append(m)
    return in_maps


def kernel(x, wq, wk, wv, wo, token_positions, _trace=False):
    x = np.asarray(x, dtype=np.float32)
    in_maps = _host_prep(x, wq, wk, wv, wo, token_positions)
    nc = _get_nc()
    res = run_bass_kernel_spmd(nc, in_maps, core_ids=list(range(NCORES)),
                               trace=_trace)
    out = np.zeros((B, S, D), np.float32)
    for b in range(B):
        acc = res.results[2 * b]["outT"].astype(np.float32) + \
            res.results[2 * b + 1]["outT"].astype(np.float32)
        out[b] = acc.T
    if _trace:
        kernel.last_results = res
    return out


# revision 17
# speedup vs baseline: 1.1837x; 1.0318x over previous
"""Trainium2 Bass kernel: causal multi-head self-attention with RoPE.

Problem: B=4, S=2048, D=1024, H=16, DK=64.  out = softmax(causal(qk^T/8)) v @ wo^T
with q,k RoPE-rotated.

Sharding: 8 cores = (batch b in 0..3) x (head-group g in 0..1, 8 heads each).
Each core computes its batch's QKV for its 8 heads, causal attention, and a
partial output projection; the host sums the two group-partials per batch.

Schedule (per core): the PE's only idle source is waiting on ACT (exp) in
attention phases, so all non-attention matmul work is used as filler there:
  upfront: pair-0 q/k projection + RoPE, v projection tiles 0..5.
  pair loop: scores (ST[k,q], K=64, two heads via tile_position row groups)
    -> exp at FD=1024 -> attn@V with ones-augmented V accumulating in PSUM;
    filler dripped between groups: remaining v-proj tiles, next pair's q/k
    proj + rope, and (during pair 3) per-qc output projection as soon as
    that qc's a_t completes.  Normalization per (pair, qc):
    reciprocal_approx_fast + broadcast-DMA.
"""
import os
import sys

for _p in ("/opt/trn_rl_repo", "/root/.axon_site/_ro/trn_rl_repo"):
    if os.path.isdir(_p) and _p not in sys.path:
        sys.path.insert(0, _p)

import numpy as np
import ml_dtypes

import concourse.bass as bass
import concourse.mybir as mybir
import concourse.tile as tile
from concourse import bacc
from concourse.bass_utils import run_bass_kernel_spmd

B, S, D, H = 4, 2048, 1024, 16
DK = D // H          # 64
HG = 8               # heads per group
NG = 2               # head groups (cores per batch)
THETA = 10000.0
NCORES = 8

BF16 = mybir.dt.bfloat16
F32 = mybir.dt.float32
bf16 = ml_dtypes.bfloat16

QT = 512             # q tile width (free dim)
NQT = S // QT        # 4
NKT = S // 128       # 16 k chunks
NJT = HG * DK // 128  # 4 j-tiles (head pairs)
NDC = D // 128       # 8 d chunks
NMT = D // 128       # 8 output m tiles
NVUP = 6             # v-proj tiles emitted before the pair loop


def _build_nc():
    nc = bacc.Bacc("TRN2", target_bir_lowering=False, debug=False)
    xT = nc.dram_tensor("xT", [D, S], BF16, kind="ExternalInput").ap()
    wqT = nc.dram_tensor("wqT", [D, HG * DK], BF16, kind="ExternalInput").ap()
    wkT = nc.dram_tensor("wkT", [D, HG * DK], BF16, kind="ExternalInput").ap()
    wvT = nc.dram_tensor("wvT", [D, HG * DK], BF16, kind="ExternalInput").ap()
    woT = nc.dram_tensor("woT", [HG * DK, D], BF16, kind="ExternalInput").ap()
    c128 = nc.dram_tensor("c128", [128, S], BF16, kind="ExternalInput").ap()
    s128 = nc.dram_tensor("s128", [128, S], BF16, kind="ExternalInput").ap()
    maskd = nc.dram_tensor("maskd", [128, 4, QT], BF16, kind="ExternalInput").ap()
    outT = nc.dram_tensor("outT", [D, S], F32, kind="ExternalOutput").ap()

    from contextlib import ExitStack
    with tile.TileContext(nc) as tc, ExitStack() as stk:
        pp = stk.enter_context(tc.tile_pool(name="persist", bufs=1))
        ep = stk.enter_context(tc.tile_pool(name="epool", bufs=6))
        sp = stk.enter_context(tc.tile_pool(name="smalls", bufs=2))
        qw = stk.enter_context(tc.tile_pool(name="qkvwork", bufs=2))
        ps_st = stk.enter_context(
            tc.tile_pool(name="ps_st", bufs=2, space="PSUM"))
        ps_ov = stk.enter_context(
            tc.tile_pool(name="ps_ov", bufs=2, space="PSUM"))
        ps_qkv = stk.enter_context(
            tc.tile_pool(name="ps_qkv", bufs=2, space="PSUM"))

        # ---------------- persistent tiles ----------------
        wo_sb = pp.tile([128, NJT, D], BF16)
        m_sb = pp.tile([128, 4, QT], BF16)
        qrot = pp.tile([128, NJT, S], BF16)
        krot = pp.tile([128, NJT, S], BF16)
        v_aug = pp.tile([128, NKT, HG, 66], BF16)
        a_t = pp.tile([128, NJT, S], BF16)
        xT_sb = pp.tile([128, NDC, S], BF16)
        wq_sb = pp.tile([128, NDC, HG * DK], BF16)
        wk_sb = pp.tile([128, NDC, HG * DK], BF16)
        wv_sb = pp.tile([128, NDC, HG * DK], BF16)
        c_sb = pp.tile([128, S], BF16)
        s_sb = pp.tile([128, S], BF16)

        nc.gpsimd.memset(v_aug[:, :, :, 64:65], 1.0)

        # ---------------- input DMAs, critical-path first ----------------
        for dc in range(NDC):
            nc.sync.dma_start(xT_sb[:, dc, :], xT[dc * 128:(dc + 1) * 128, :])
        for dc in range(NDC):
            nc.sync.dma_start(wq_sb[:, dc, :], wqT[dc * 128:(dc + 1) * 128, :])
        nc.sync.dma_start(c_sb[:], c128[:])
        nc.sync.dma_start(s_sb[:], s128[:])
        for dc in range(NDC):
            nc.sync.dma_start(wk_sb[:, dc, :], wkT[dc * 128:(dc + 1) * 128, :])
        for dc in range(NDC):
            nc.sync.dma_start(wv_sb[:, dc, :], wvT[dc * 128:(dc + 1) * 128, :])
        nc.sync.dma_start(m_sb[:], maskd[:])
        for jc in range(NJT):
            nc.sync.dma_start(wo_sb[:, jc, :], woT[jc * 128:(jc + 1) * 128, :])

        # ---------------- unit emitters ----------------
        def vproj_unit(tt):
            ps = ps_qkv.tile([128, QT], F32, tag="qv", name=f"psv{tt}")
            for dc in range(NDC):
                nc.tensor.matmul(
                    ps[:],
                    xT_sb[:, dc, tt * 128:(tt + 1) * 128],
                    wv_sb[:, dc, :],
                    start=(dc == 0), stop=(dc == NDC - 1))
            nc.vector.tensor_copy(
                v_aug[:, tt, :, 0:64],
                ps[:].rearrange("p (h d) -> p h d", h=HG))

        def proj_unit(pair, name, w_sb, pre, tn):
            ps = ps_qkv.tile([128, QT], F32, tag="qv",
                             name=f"ps{name}{pair}{tn}")
            for dc in range(NDC):
                nc.tensor.matmul(
                    ps[:],
                    w_sb[:, dc, pair * 128:(pair + 1) * 128],
                    xT_sb[:, dc, tn * QT:(tn + 1) * QT],
                    start=(dc == 0), stop=(dc == NDC - 1))
            nc.vector.tensor_copy(pre[:, tn * QT:(tn + 1) * QT], ps[:])

        def rope_unit(pair, name, pre, dst):
            swp = qw.tile([128, S], BF16, tag="swp", name=f"swp{name}{pair}")
            for a in range(4):
                lo, sw = 32 * a, 32 * (a ^ 1)
                nc.sync.dma_start(swp[lo:lo + 32, :], pre[sw:sw + 32, :])
            nc.vector.tensor_mul(dst[:, pair, :], pre[:], c_sb[:])
            nc.vector.tensor_mul(swp[:], swp[:], s_sb[:])
            nc.vector.tensor_add(dst[:, pair, :], dst[:, pair, :], swp[:])

        def _diag_off(qc, g, half):
            """first unmasked q column of this kc chunk within the q tile
            (0 for interior chunks; 128*j for the j-th diagonal chunk)"""
            par = g - 2 * qc
            if par < 0:
                return 0
            return 128 * (2 * par + half)

        def emit_scores(pair, qc, g):
            # both heads in ONE 4-bank PSUM tile so exp runs as a single
            # wide ACT instruction (ACT has ~0.5us fixed cost per instr)
            st = ps_st.tile([128, 2, 2 * QT], F32, tag="st", bufs=1,
                            name=f"st_{pair}{qc}{g}")
            for half in range(2):
                kc = 2 * g + half
                off = _diag_off(qc, g, half)
                for h01 in range(2):
                    lo = 64 * h01
                    nc.tensor.matmul(
                        st[:, h01, half * QT + off:(half + 1) * QT],
                        krot[lo:lo + 64, pair, kc * 128:(kc + 1) * 128],
                        qrot[lo:lo + 64, pair, qc * QT + off:(qc + 1) * QT],
                        start=True, stop=True,
                        tile_position=(lo, 0))
            return st

        def emit_tail(pair, qc, g, st, ov0, ov1, last):
            par = g - 2 * qc
            e = ep.tile([128, 2, 2 * QT], BF16, tag="e", bufs=3,
                        name=f"e{pair}{qc}{g}")
            if par < 0:
                nc.scalar.activation(
                    e[:], st[:], mybir.ActivationFunctionType.Exp,
                    scale=0.125)
            else:
                # diagonal: exp only the unmasked q ranges (both heads in
                # one instr), then mask the 128-wide triangle band per chunk
                for half in range(2):
                    off = _diag_off(qc, g, half)
                    sl = slice(half * QT + off, (half + 1) * QT)
                    nc.scalar.activation(
                        e[:, :, sl], st[:, :, sl],
                        mybir.ActivationFunctionType.Exp, scale=0.125)
                    for h01 in range(2):
                        nc.vector.tensor_mul(
                            e[:, h01, half * QT + off:half * QT + off + 128],
                            e[:, h01, half * QT + off:half * QT + off + 128],
                            m_sb[:, 2 * par + half, off:off + 128])
            for h01, ov in ((0, ov0), (1, ov1)):
                for half in range(2):
                    kc = 2 * g + half
                    off = _diag_off(qc, g, half)
                    nc.tensor.matmul(
                        ov[:, off:QT] if off else ov[:],
                        v_aug[:, kc, 2 * pair + h01, 0:65],
                        e[:, h01, half * QT + off:(half + 1) * QT],
                        start=(kc == 0),
                        stop=(last and half == 1))

        def emit_evac(pair, qc, ov0, ov1):
            """Part A: free the ov PSUM banks and stage the denominators."""
            den = sp.tile([2, QT], F32, tag="den", bufs=3,
                          name=f"den{pair}{qc}")
            ous = []
            for h01, ov in ((0, ov0), (1, ov1)):
                ou = ep.tile([65, QT], BF16, tag="ou", bufs=6,
                             name=f"ou{pair}{qc}{h01}")
                nc.vector.tensor_copy(ou[:], ov[:])
                nc.gpsimd.dma_start(den[h01:h01 + 1, :], ou[64:65, :])
                ous.append(ou)
            return den, ous

        def emit_recip(pair, qc, den):
            """Part B1: reciprocal + broadcast DMAs (deferred one qc)."""
            recip = sp.tile([2, QT], F32, tag="recip", name=f"rcp{pair}{qc}")
            nc.vector.reciprocal_approx_fast(recip[:], den[:])
            rbs = []
            for h01 in range(2):
                rb = sp.tile([64, QT], BF16, tag="rb", bufs=5,
                             name=f"rb{pair}{qc}{h01}")
                nc.gpsimd.dma_start(
                    rb[:],
                    recip[h01:h01 + 1, :]
                    .unsqueeze(1).to_broadcast((1, 64, QT)))
                rbs.append(rb)
            return rbs

        def emit_div(pair, qc, ous, rbs):
            """Part B2: the normalize multiplies (deferred further)."""
            nc.vector.tensor_mul(
                a_t[0:64, pair, qc * QT:(qc + 1) * QT],
                ous[0][0:64, :], rbs[0][:])
            an = sp.tile([64, QT], BF16, tag="an", bufs=3,
                         name=f"an{pair}{qc}")
            nc.vector.tensor_mul(an[:], ous[1][0:64, :], rbs[1][:])
            nc.sync.dma_start(
                a_t[64:128, pair, qc * QT:(qc + 1) * QT], an[:])

        def outproj_unit(qc, mt):
            op = ps_qkv.tile([128, QT], F32, tag="qv", name=f"op{qc}{mt}")
            for jc in range(NJT):
                nc.tensor.matmul(
                    op[:],
                    wo_sb[:, jc, mt * 128:(mt + 1) * 128],
                    a_t[:, jc, qc * QT:(qc + 1) * QT],
                    start=(jc == 0), stop=(jc == NJT - 1))
            ot = sp.tile([128, QT], F32, tag="ot", bufs=3,
                         name=f"ot{qc}{mt}")
            nc.vector.tensor_copy(ot[:], op[:])
            # scalar queue: keep outT stores off the sync queue, which
            # carries the a_t writes that gate the remaining outproj work
            nc.scalar.dma_start(
                outT[mt * 128:(mt + 1) * 128, qc * QT:(qc + 1) * QT],
                ot[:])

        def proj_units(pair):
            preq = qw.tile([128, S], BF16, tag="preq", name=f"preq{pair}")
            prek = qw.tile([128, S], BF16, tag="prek", name=f"prek{pair}")
            for tn in range(NQT):
                yield lambda tn=tn: proj_unit(pair, "q", wq_sb, preq, tn)
            yield lambda: rope_unit(pair, "q", preq, qrot)
            for tn in range(NQT):
                yield lambda tn=tn: proj_unit(pair, "k", wk_sb, prek, tn)
            yield lambda: rope_unit(pair, "k", prek, krot)

        # ---------------- upfront: pair-0 q/k + rope, v tiles 0..5 --------
        for u in proj_units(0):
            u()
        for tt in range(NVUP):
            vproj_unit(tt)

        from collections import deque
        filler = deque()
        for tt in range(NVUP, NKT):
            filler.append(lambda tt=tt: vproj_unit(tt))

        # per qc: how many filler units to drip in after each group
        # (placed mid-stream so the scores pipeline stays primed)
        UNIT_BUDGET = {0: 3, 1: 5, 2: 4, 3: 2}
        UNIT_BUDGET_P3 = {0: 2, 1: 4, 2: 6, 3: 8}


        for pair in range(NJT):
            if pair + 1 < NJT:
                filler.extend(proj_units(pair + 1))
            budgets = UNIT_BUDGET_P3 if pair == NJT - 1 else UNIT_BUDGET
            for qc in range(NQT):
                ngrp = 2 * qc + 2
                ov0 = ps_ov.tile([65, QT], F32, tag="ov",
                                 name=f"ov0_{pair}{qc}")
                ov1 = ps_ov.tile([65, QT], F32, tag="ov",
                                 name=f"ov1_{pair}{qc}")
                budget = budgets[qc]
                pend = None
                for g in range(ngrp):
                    sts = emit_scores(pair, qc, g)
                    if pend is not None:
                        pg, p0 = pend
                        emit_tail(pair, qc, pg, p0, ov0, ov1, last=False)
                    pend = (g, sts)
                    if g >= 1 and budget > 0 and filler:
                        for _ in range(min(2, budget, len(filler))):
                            filler.popleft()()
                            budget -= 1
                pg, p0 = pend
                emit_tail(pair, qc, pg, p0, ov0, ov1, last=True)
                den, ous = emit_evac(pair, qc, ov0, ov1)
                rbs = emit_recip(pair, qc, den)
                emit_div(pair, qc, ous, rbs)
                if pair == NJT - 1:
                    for mt in range(NMT):
                        filler.append(
                            lambda qc=qc, mt=mt: outproj_unit(qc, mt))

            if pair < NJT - 1:
                while filler:
                    filler.popleft()()

        while filler:
            filler.popleft()()

    nc.compile()
    return nc


_NC_CACHE = {}


def _get_nc():
    if "nc" not in _NC_CACHE:
        _NC_CACHE["nc"] = _build_nc()
    return _NC_CACHE["nc"]


def _host_prep(x, wq, wk, wv, wo, token_positions):
    head_perm = np.concatenate([np.arange(0, DK, 2), np.arange(1, DK, 2)])
    pos = np.asarray(token_positions).astype(np.float32)
    half = np.arange(0, DK, 2, dtype=np.float32) / DK
    inv_freq = THETA ** (-half)
    ang = pos[:, None] * inv_freq[None, :]        # [S, 32]
    cosT = np.cos(ang).T.astype(np.float32)       # [32, S]
    sinT = np.sin(ang).T.astype(np.float32)
    c128 = np.tile(cosT, (4, 1)).astype(bf16)     # [128, S]
    s128 = np.concatenate([-sinT, sinT, -sinT, sinT], 0).astype(bf16)

    kp = np.arange(128)[:, None, None]
    jj = np.arange(4)[None, :, None]
    qf = np.arange(QT)[None, None, :]
    maskd = (qf >= kp + 128 * jj).astype(bf16)    # [128, 4, QT]

    def prep_qk(w, g):
        rows = w.reshape(H, DK, D)[g * HG:(g + 1) * HG][:, head_perm]
        return np.ascontiguousarray(rows.reshape(HG * DK, D).T).astype(bf16)

    def prep_v(w, g):
        rows = w.reshape(H, DK, D)[g * HG:(g + 1) * HG]
        return np.ascontiguousarray(rows.reshape(HG * DK, D).T).astype(bf16)

    common = {"c128": c128, "s128": s128, "maskd": maskd}
    in_maps = []
    for c in range(NCORES):
        b, g = c // NG, c % NG
        m = dict(common)
        m["xT"] = np.ascontiguousarray(x[b].T).astype(bf16)
        m["wqT"] = prep_qk(wq, g)
        m["wkT"] = prep_qk(wk, g)
        m["wvT"] = prep_v(wv, g)
        m["woT"] = np.ascontiguousarray(wo[:, g * HG * DK:(g + 1) * HG * DK].T
                                        ).astype(bf16)
        in_maps.append(m)
    return in_maps


def kernel(x, wq, wk, wv, wo, token_positions, _trace=False):
    x = np.asarray(x, dtype=np.float32)
    in_maps = _host_prep(x, wq, wk, wv, wo, token_positions)
    nc = _get_nc()
    res = run_bass_kernel_spmd(nc, in_maps, core_ids=list(range(NCORES)),
                               trace=_trace)
    out = np.zeros((B, S, D), np.float32)
    for b in range(B):
        acc = res.results[2 * b]["outT"].astype(np.float32) + \
            res.results[2 * b + 1]["outT"].astype(np.float32)
        out[b] = acc.T
    if _trace:
        kernel.last_results = res
    return out
